# revision 4
# baseline (speedup 1.0000x reference)
"""CAMMambaBlock Trainium2 kernel, v2.

Data-parallel over batch: 8 batch elements -> 8 NeuronCores. Each core runs
the full block on its (c=128, L=9216) slice, streaming over L in 6 chunks of
1536.

Key structure: the per-state mults are UNFUSED from the scan chain.
u_n = v*B_n depends only on prefix outputs, so all 16 scans run
back-to-back on DVE with no interleaved serial mults; p_n = h_n*C_n runs
after scan_n and feeds PE identity-matmul ysum accumulation. Silu/softplus
use direct activation functions (2 act-table swaps per chunk); ln_w/ln_b
are folded into the in_proj weights/biases host-side so LN is two DVE ops.
Cross-chunk software pipelining via emission hooks inside the scan loop.
"""
import types
import numpy as np
import ml_dtypes
from contextlib import ExitStack

import bass_rust

import concourse.bass as bass
import concourse.bacc as bacc
import concourse.tile as tile
from concourse import mybir
from concourse.bass_utils import run_bass_kernel_spmd
from concourse.hw_specs import get_activation_tables


def _two_act_tables(self):
    """Limit activation tables to the two we batch by, so the table-load
    pass inserts at most one swap per batch."""
    if not any(i.opcode == "Activation" for i in self.all_instructions()):
        return
    keep = ("natural_log_exp_and_others", "silu_and_others")
    tables = [(n, (f if n in keep else set()))
              for n, f in get_activation_tables(self.m.arch).items()]
    bass_rust.insert_act_table_loads(self, tables)

F32 = mybir.dt.float32
BF16 = mybir.dt.bfloat16
AF = mybir.ActivationFunctionType
OP = mybir.AluOpType

C = 128
NSTATE = 16
RANK = 8
LN_EPS = 1e-5
DCONV = 4

L_FULL = 96 * 96

# states whose p-mult runs on the Pool engine. Measured: Pool shares an SBUF
# port with DVE 2-port ops, so Pool compute during scans halves BOTH engines'
# throughput. Keep empty.
POOL_P = ()


def build_nc(L, Tc, sub=512):
    assert L % Tc == 0 and Tc % sub == 0
    nchunk = L // Tc
    nsub = Tc // sub

    nc = bacc.Bacc()
    x_in = nc.declare_dram_parameter("x", [C, L], F32, isOutput=False)
    w_inT = nc.declare_dram_parameter("w_inT", [C, 5 * C], BF16, isOutput=False)
    w_xpT = nc.declare_dram_parameter("w_xpT", [C, RANK + 2 * NSTATE], BF16,
                                      isOutput=False)
    w_dtT = nc.declare_dram_parameter("w_dtT", [RANK, C], BF16, isOutput=False)
    w_outT = nc.declare_dram_parameter("w_outT", [C, C], BF16, isOutput=False)
    ident = nc.declare_dram_parameter("ident", [C, C], BF16, isOutput=False)
    nident = nc.declare_dram_parameter("nident", [C, C], BF16, isOutput=False)
    diag_d_in = nc.declare_dram_parameter("diag_d", [C, C], BF16,
                                          isOutput=False)
    identf = nc.declare_dram_parameter("identf", [C, C], F32, isOutput=False)
    # per-partition columns:
    # [ln_w, ln_b, conv_b, dt_b, D, unused*4, eps, -conv_b]
    cols = nc.declare_dram_parameter("cols", [C, 13], F32, isOutput=False)
    a_cols = nc.declare_dram_parameter("a_cols", [C, NSTATE], F32,
                                       isOutput=False)
    y_out = nc.declare_dram_parameter("y", [C, L], F32, isOutput=True)

    with tile.TileContext(nc) as tc, ExitStack() as ctx:
        wpool = ctx.enter_context(tc.tile_pool(name="weights", bufs=1))
        io = ctx.enter_context(tc.tile_pool(name="io", bufs=2))
        sqp = ctx.enter_context(tc.tile_pool(name="sqp", bufs=1))
        lnp = ctx.enter_context(tc.tile_pool(name="lnp", bufs=1))
        unp = ctx.enter_context(tc.tile_pool(name="unp", bufs=2))
        gate = ctx.enter_context(tc.tile_pool(name="gate", bufs=2))
        dtvp = ctx.enter_context(tc.tile_pool(name="dtvp", bufs=2))
        bctp = ctx.enter_context(tc.tile_pool(name="bctp", bufs=2))
        scr = ctx.enter_context(tc.tile_pool(name="scr", bufs=2))
        scr0 = ctx.enter_context(tc.tile_pool(name="scr0", bufs=1))
        dap = ctx.enter_context(tc.tile_pool(name="dap", bufs=9))
        bcrp = ctx.enter_context(tc.tile_pool(name="bcrp", bufs=6))
        b0p = ctx.enter_context(tc.tile_pool(name="b0p", bufs=2))
        up = ctx.enter_context(tc.tile_pool(name="up", bufs=4))
        hp = ctx.enter_context(tc.tile_pool(name="hp", bufs=3))
        pp = ctx.enter_context(tc.tile_pool(name="pp", bufs=3))
        ygp = ctx.enter_context(tc.tile_pool(name="ygp", bufs=2))
        state = ctx.enter_context(tc.tile_pool(name="state", bufs=1))
        dram = ctx.enter_context(tc.tile_pool(name="dram", bufs=2,
                                              space="DRAM"))
        ps_st = ctx.enter_context(tc.tile_pool(name="ps_st", bufs=1,
                                               space="PSUM"))
        ps_b = ctx.enter_context(tc.tile_pool(name="ps_b", bufs=2,
                                              space="PSUM"))
        ps_y = ctx.enter_context(tc.tile_pool(name="ps_y", bufs=1,
                                              space="PSUM"))

        # ---- chunk-0 input first: nothing else gates the LN stats ----
        xin0 = io.tile([C, Tc], F32, tag="xin", name="xin0")
        nc.sync.dma_start(xin0[:], x_in[:, 0:Tc])
        xinbf0 = io.tile([C, Tc], BF16, tag="xinbf", name="xinbf0")
        nc.vector.tensor_copy(xinbf0[:], xin0[:])
        P0 = {"xin": xin0, "xinbf": xinbf0}

        # ---- weights ----
        ones_c = wpool.tile([C, C], BF16, tag="ones")
        nc.gpsimd.memset(ones_c[:], 1.0 / C)
        winT = wpool.tile([C, 5 * C], BF16, tag="winT")
        nc.sync.dma_start(winT[:], w_inT[:])
        wxpT = wpool.tile([C, RANK + 2 * NSTATE], BF16, tag="wxpT")
        nc.sync.dma_start(wxpT[:], w_xpT[:])
        wdtT = wpool.tile([RANK, C], BF16, tag="wdtT")
        nc.sync.dma_start(wdtT[:], w_dtT[:])
        woutT = wpool.tile([C, C], BF16, tag="woutT")
        nc.sync.dma_start(woutT[:], w_outT[:])
        idn = wpool.tile([C, C], BF16, tag="idn")
        nc.sync.dma_start(idn[:], ident[:])
        nidn = wpool.tile([C, C], BF16, tag="nidn")
        nc.sync.dma_start(nidn[:], nident[:])
        diag_d = wpool.tile([C, C], BF16, tag="diag_d")
        nc.sync.dma_start(diag_d[:], diag_d_in[:])
        idnf = wpool.tile([C, C], F32, tag="idnf")
        nc.sync.dma_start(idnf[:], identf[:])
        colsb = wpool.tile([C, 13], F32, tag="cols")
        nc.sync.dma_start(colsb[:], cols[:])
        acol = wpool.tile([C, NSTATE], F32, tag="acol")
        nc.sync.dma_start(acol[:], a_cols[:])
        conv_b, dt_b = colsb[:, 2:3], colsb[:, 3:4]
        zb_col = colsb[:, 5:6]
        corr_cols = colsb[:, 6:9]
        eps_col = colsb[:, 9:10]
        ncb_col = colsb[:, 10:11]
        nzb_col = colsb[:, 11:12]

        carries = [state.tile([C, 1], BF16, tag=f"carry{n}",
                              name=f"carry{n}")
                   for n in range(NSTATE)]

        P = {}  # per-chunk produced tiles

        def emit_in_dma(k):
            t0 = k * Tc
            d = P.setdefault(k, {})
            d["xin"] = io.tile([C, Tc], F32, tag="xin", name="xin")
            nc.sync.dma_start(d["xin"][:], x_in[:, t0:t0 + Tc])
            d["xinbf"] = io.tile([C, Tc], BF16, tag="xinbf", name="xinbf")
            nc.gpsimd.dma_start(d["xinbf"][:], x_in[:, t0:t0 + Tc])

        def emit_ln_a(k):
            d = P[k]
            sq = sqp.tile([C, Tc], BF16, tag="sq", name="sq")
            nc.scalar.activation(sq[:], d["xin"][:], AF.Square)
            mub = lnp.tile([C, Tc], BF16, tag="mub", name="mub")
            rstd = lnp.tile([C, Tc], BF16, tag="rstd", name="rstd")
            for j in range(nsub):
                sl = slice(j * sub, (j + 1) * sub)
                mu = ps_st.tile([C, sub], F32, tag="mu", name="mu")
                nc.tensor.matmul(mu[:], ones_c[:], d["xinbf"][:, sl],
                                 start=True, stop=True)
                musq = scr.tile([C, sub], BF16, tag="musq", name="musq")
                nc.scalar.activation(musq[:], mu[:], AF.Square)
                nc.scalar.copy(mub[:, sl], mu[:])
                var = ps_st.tile([C, sub], F32, tag="m2", name="var")
                nc.tensor.matmul(var[:], ones_c[:], sq[:, sl],
                                 start=True, stop=False)
                nc.tensor.matmul(var[:], nidn[:], musq[:],
                                 start=False, stop=True)
                lnv = scr0.tile([C, sub], F32, tag="lnv", name="lnv")
                nc.scalar.activation(lnv[:], var[:], AF.Ln, bias=eps_col)
                nc.scalar.activation(rstd[:, sl], lnv[:], AF.Exp, scale=-0.5)
            d["mub"], d["rstd"] = mub, rstd

        def emit_ln_b(k):
            d = P[k]
            # ln_w is folded into the in_proj weights and ln_b into the
            # conv/z biases (host-side), so un is just (x-mu)*rstd. Halo
            # lives at cols 1..3; data at 4..Tc+3 (4B-aligned write).
            un = unp.tile([C, Tc + DCONV], BF16, tag="un", name="un")
            if k == 0:
                nc.vector.memset(un[:, 1:DCONV], 0.0)
            else:
                nc.vector.tensor_copy(un[:, 1:DCONV],
                                      P[k - 1]["un"][:, Tc + 1:Tc + DCONV])
            if k == 0:
                for j in range(nsub):
                    sl = slice(j * sub, (j + 1) * sub)
                    dmu = scr.tile([C, sub], BF16, tag="dmu0", name="dmu")
                    nc.vector.tensor_tensor(dmu[:], d["xinbf"][:, sl],
                                            d["mub"][:, sl], OP.subtract)
                    nc.vector.tensor_tensor(
                        un[:, DCONV + j * sub:DCONV + (j + 1) * sub],
                        dmu[:], d["rstd"][:, sl], OP.mult)
            else:
                dmu = scr.tile([C, Tc], BF16, tag="dmu", name="dmu")
                nc.vector.tensor_tensor(dmu[:], d["xinbf"][:], d["mub"][:],
                                        OP.subtract)
                nc.vector.tensor_tensor(un[:, DCONV:Tc + DCONV],
                                        dmu[:], d["rstd"][:], OP.mult)
            d["un"] = un

        def emit_conv(k):
            d = P[k]
            un = d["un"]
            xs = gate.tile([C, Tc], BF16, tag="xs", name="xs")
            zs = gate.tile([C, Tc], BF16, tag="zs", name="zs")
            for j in range(nsub):
                sl = slice(j * sub, (j + 1) * sub)
                xm_ps = ps_b.tile([C, sub], F32, tag="mmb", name="xm_ps")
                for kk in range(DCONV):
                    nc.tensor.matmul(
                        xm_ps[:], winT[:, kk * C:(kk + 1) * C],
                        un[:, kk + 1 + j * sub:kk + 1 + j * sub + sub],
                        start=(kk == 0), stop=(kk == DCONV - 1))
                nc.scalar.activation(xs[:, sl], xm_ps[:], AF.Silu,
                                     bias=conv_b)
            for j in range(nsub):
                sl = slice(j * sub, (j + 1) * sub)
                z_ps = ps_b.tile([C, sub], F32, tag="mmb", name="z_ps")
                nc.tensor.matmul(z_ps[:], winT[:, 4 * C:5 * C],
                                 un[:, DCONV + j * sub:
                                     DCONV + j * sub + sub],
                                 start=True, stop=True)
                nc.scalar.activation(zs[:, sl], z_ps[:], AF.Silu,
                                     bias=zb_col)
            d["xs"], d["zs"] = xs, zs

        def emit_proj(k):
            d = P[k]
            xs = d["xs"]
            bcdt = bctp.tile([2 * NSTATE + RANK, Tc], BF16, tag="bcdt",
                             name="bcdt")
            for j in range(nsub):
                sl = slice(j * sub, (j + 1) * sub)
                dblt = ps_b.tile([C, sub], F32, tag="mmb", name="dblt")
                nc.tensor.matmul(dblt[0:RANK + 2 * NSTATE, :], wxpT[:],
                                 xs[:, sl], start=True, stop=True)
                if k == 0:
                    nc.vector.tensor_copy(bcdt[:, sl],
                                          dblt[0:2 * NSTATE + RANK, :])
                else:
                    nc.scalar.copy(bcdt[:, sl],
                                   dblt[0:2 * NSTATE + RANK, :])
            bcd = dram.tile([NSTATE, 2 * Tc], BF16, tag="bcd", name="bcd")
            # B_0 per-sub first so the first u-mult can start early
            for j in range(nsub):
                nc.sync.dma_start(bcd[0:1, j * sub:(j + 1) * sub],
                                  bcdt[RANK:RANK + 1, j * sub:(j + 1) * sub])
            nc.sync.dma_start(bcd[0:1, Tc:2 * Tc],
                              bcdt[RANK + 1:RANK + 2, :])
            nc.sync.dma_start(bcd[1:NSTATE, :],
                              bcdt[RANK + 2:RANK + 2 * NSTATE, :])
            d["bcd"] = bcd

            dt_bf = dtvp.tile([C, Tc], BF16, tag="dt", name="dt")
            for j in range(nsub):
                sl = slice(j * sub, (j + 1) * sub)
                dt_ps = ps_b.tile([C, sub], F32, tag="mmb", name="dt_ps")
                nc.tensor.matmul(dt_ps[:], wdtT[:], bcdt[0:RANK, sl],
                                 start=True, stop=True)
                spe = scr0.tile([C, sub], F32, tag="spe", name="spe")
                nc.scalar.activation(spe[:], dt_ps[:], AF.Exp, bias=dt_b)
                nc.scalar.activation(dt_bf[:, sl], spe[:], AF.Ln, bias=1.0)
            d["dt"] = dt_bf

        def emit_dA(k):
            d = P[k]
            dAs = []
            if k == 0:
                dA0 = dap.tile([C, Tc], BF16, tag="dA", name="dA")
                for j in range(nsub):
                    sl = slice(j * sub, (j + 1) * sub)
                    nc.scalar.activation(dA0[:, sl], d["dt"][:, sl],
                                         AF.Exp, scale=acol[:, 0:1])
                dAs.append(dA0)
            for n in range(len(dAs), NSTATE):
                dA = dap.tile([C, Tc], BF16, tag="dA", name="dA")
                nc.scalar.activation(dA[:], d["dt"][:], AF.Exp,
                                     scale=acol[:, n:n + 1])
                dAs.append(dA)
            d["dAs"] = dAs
            # broadcast DMAs: B_0, then pairs (C_n | B_{n+1}), then C_15
            bcd = d["bcd"]
            bcdf = bcd.tensor.reshape([1, NSTATE * 2 * Tc])
            b0 = b0p.tile([C, Tc], BF16, tag="b0", name="b0")
            for j in range(nsub):
                sl = slice(j * sub, (j + 1) * sub)
                nc.sync.dma_start(b0[:, sl],
                                  bcd[0:1, sl].broadcast_to([C, sub]))
            d["b0"] = b0
            prs = []
            for n in range(NSTATE - 1):
                pr = bcrp.tile([C, 2 * Tc], BF16, tag="bcr", name="bcr")
                nc.sync.dma_start(
                    pr[:],
                    bcdf[0:1, (2 * n + 1) * Tc:(2 * n + 3) * Tc]
                    .broadcast_to([C, 2 * Tc]))
                prs.append(pr)
            c15 = b0p.tile([C, Tc], BF16, tag="c15", name="c15")
            nc.sync.dma_start(
                c15[:],
                bcdf[0:1, (2 * NSTATE - 1) * Tc:2 * NSTATE * Tc]
                .broadcast_to([C, Tc]))
            d["prs"], d["c15"] = prs, c15

        def emit_v(k):
            d = P[k]
            v = dtvp.tile([C, Tc], BF16, tag="v", name="v")
            nc.vector.tensor_tensor(v[:], d["dt"][:], d["xs"][:], OP.mult)
            d["v"] = v
            d["us"] = [None] * NSTATE

        def emit_u(k, n):
            d = P[k]
            u = up.tile([C, Tc], BF16, tag="u", name="u")
            if n == 0:
                nc.vector.tensor_tensor(u[:], d["v"][:], d["b0"][:], OP.mult)
            else:
                nc.vector.tensor_tensor(u[:], d["v"][:],
                                        d["prs"][n - 1][:, Tc:2 * Tc],
                                        OP.mult)
            d["us"][n] = u

        def emit_readout(k):
            d = P[k]
            t0 = k * Tc
            ysum, zs, xin = d["ysum"], d["zs"], d["xin"]
            for j in range(nsub):
                sl = slice(j * sub, (j + 1) * sub)
                yc = ygp.tile([C, sub], BF16, tag="yc", name="yc")
                nc.scalar.copy(yc[:], ysum[:, sl])
                yg = ygp.tile([C, sub], BF16, tag="yg", name="yg")
                nc.vector.tensor_tensor(yg[:], yc[:], zs[:, sl], OP.mult)
                o_ps = ps_b.tile([C, sub], F32, tag="mmb", name="o_ps")
                nc.tensor.matmul(o_ps[:], woutT[:], yg[:],
                                 start=True, stop=False)
                nc.tensor.matmul(o_ps[:], idnf[:], xin[:, sl],
                                 start=False, stop=True)
                ob = ygp.tile([C, sub], F32, tag="ob", name="ob")
                nc.scalar.copy(ob[:], o_ps[:])
                nc.sync.dma_start(y_out[:, t0 + j * sub:t0 + (j + 1) * sub],
                                  ob[:])

        def emit_prefix0():
            """Chunk-0 prefix, per-sub pipelined so the first scan starts
            ~25us earlier. Silu synthesized from exp/ln (no table swap in
            the ramp). State-0's scan runs as chained sub-scans emitted
            inline; states 0/1 dA and u_0 are produced per-sub."""
            d = P[0]
            sq = sqp.tile([C, Tc], BF16, tag="sq", name="sq")
            mub = lnp.tile([C, Tc], BF16, tag="mub", name="mub")
            rstd = lnp.tile([C, Tc], BF16, tag="rstd", name="rstd")
            un = unp.tile([C, Tc + DCONV], BF16, tag="un", name="un")
            nc.vector.memset(un[:, 1:DCONV], 0.0)
            xs = gate.tile([C, Tc], BF16, tag="xs", name="xs")
            zs = gate.tile([C, Tc], BF16, tag="zs", name="zs")
            bcdt = bctp.tile([2 * NSTATE + RANK, Tc], BF16, tag="bcdt",
                             name="bcdt")
            bcd = dram.tile([NSTATE, 2 * Tc], BF16, tag="bcd", name="bcd")
            dt_bf = dtvp.tile([C, Tc], BF16, tag="dt", name="dt")
            v = dtvp.tile([C, Tc], BF16, tag="v", name="v")
            dA0 = dap.tile([C, Tc], BF16, tag="dA", name="dA0")
            dA1 = dap.tile([C, Tc], BF16, tag="dA", name="dA1")
            b0 = b0p.tile([C, Tc], BF16, tag="b0", name="b0")
            u0 = up.tile([C, Tc], BF16, tag="u", name="u0")
            h0 = hp.tile([C, Tc], BF16, tag="h", name="h0")
            for j in range(nsub):
                sl = slice(j * sub, (j + 1) * sub)
                nc.scalar.activation(sq[:, sl], d["xin"][:, sl], AF.Square)
                mu = ps_st.tile([C, sub], F32, tag="mu", name="mu")
                nc.tensor.matmul(mu[:], ones_c[:], d["xinbf"][:, sl],
                                 start=True, stop=True)
                musq = scr.tile([C, sub], BF16, tag="musq", name="musq")
                nc.scalar.activation(musq[:], mu[:], AF.Square)
                nc.scalar.copy(mub[:, sl], mu[:])
                var = ps_st.tile([C, sub], F32, tag="m2", name="var")
                nc.tensor.matmul(var[:], ones_c[:], sq[:, sl],
                                 start=True, stop=False)
                nc.tensor.matmul(var[:], nidn[:], musq[:],
                                 start=False, stop=True)
                lnv = scr0.tile([C, sub], F32, tag="lnv", name="lnv")
                nc.scalar.activation(lnv[:], var[:], AF.Ln, bias=eps_col)
                nc.scalar.activation(rstd[:, sl], lnv[:], AF.Exp, scale=-0.5)
                dmu = scr.tile([C, sub], BF16, tag="dmu", name="dmu")
                nc.vector.tensor_tensor(dmu[:], d["xinbf"][:, sl],
                                        mub[:, sl], OP.subtract)
                nc.vector.tensor_tensor(
                    un[:, DCONV + j * sub:DCONV + (j + 1) * sub],
                    dmu[:], rstd[:, sl], OP.mult)
                xm_ps = ps_b.tile([C, sub], F32, tag="mmb", name="xm_ps")
                for kk in range(DCONV):
                    nc.tensor.matmul(
                        xm_ps[:], winT[:, kk * C:(kk + 1) * C],
                        un[:, kk + 1 + j * sub:kk + 1 + j * sub + sub],
                        start=(kk == 0), stop=(kk == DCONV - 1))
                ec1 = scr0.tile([C, sub], F32, tag="ec1", name="ec1")
                nc.scalar.activation(ec1[:], xm_ps[:], AF.Exp, scale=-1.0,
                                     bias=ncb_col)
                ec2 = scr0.tile([C, sub], F32, tag="ec2", name="ec2")
                nc.scalar.activation(ec2[:], ec1[:], AF.Ln, bias=1.0)
                sgc = scr0.tile([C, sub], BF16, tag="sgc", name="sgc")
                nc.scalar.activation(sgc[:], ec2[:], AF.Exp, scale=-1.0)
                xmb = scr0.tile([C, sub], BF16, tag="xmb", name="xmb")
                nc.scalar.activation(xmb[:], xm_ps[:], AF.Identity,
                                     bias=conv_b)
                nc.vector.tensor_tensor(xs[:, sl], xmb[:], sgc[:], OP.mult)
                z_ps = ps_b.tile([C, sub], F32, tag="mmb", name="z_ps")
                nc.tensor.matmul(z_ps[:], winT[:, 4 * C:5 * C],
                                 un[:, DCONV + j * sub:
                                     DCONV + j * sub + sub],
                                 start=True, stop=True)
                es1 = scr0.tile([C, sub], F32, tag="es1", name="es1")
                nc.scalar.activation(es1[:], z_ps[:], AF.Exp, scale=-1.0,
                                     bias=nzb_col)
                es2 = scr0.tile([C, sub], F32, tag="es2", name="es2")
                nc.scalar.activation(es2[:], es1[:], AF.Ln, bias=1.0)
                sgz = scr0.tile([C, sub], BF16, tag="sgz", name="sgz")
                nc.scalar.activation(sgz[:], es2[:], AF.Exp, scale=-1.0)
                zmb = scr0.tile([C, sub], BF16, tag="zmb", name="zmb")
                nc.scalar.activation(zmb[:], z_ps[:], AF.Identity,
                                     bias=zb_col)
                nc.vector.tensor_tensor(zs[:, sl], zmb[:], sgz[:], OP.mult)
                dblt = ps_b.tile([C, sub], F32, tag="mmb", name="dblt")
                nc.tensor.matmul(dblt[0:RANK + 2 * NSTATE, :], wxpT[:],
                                 xs[:, sl], start=True, stop=True)
                if k == 0:
                    nc.vector.tensor_copy(bcdt[:, sl],
                                          dblt[0:2 * NSTATE + RANK, :])
                else:
                    nc.scalar.copy(bcdt[:, sl],
                                   dblt[0:2 * NSTATE + RANK, :])
                nc.sync.dma_start(bcd[0:1, sl],
                                  bcdt[RANK:RANK + 1, sl])
                nc.sync.dma_start(b0[:, sl],
                                  bcd[0:1, sl].broadcast_to([C, sub]))
                dt_ps = ps_b.tile([C, sub], F32, tag="mmb", name="dt_ps")
                nc.tensor.matmul(dt_ps[:], wdtT[:], bcdt[0:RANK, sl],
                                 start=True, stop=True)
                spe = scr0.tile([C, sub], F32, tag="spe", name="spe")
                nc.scalar.activation(spe[:], dt_ps[:], AF.Exp, bias=dt_b)
                nc.scalar.activation(dt_bf[:, sl], spe[:], AF.Ln, bias=1.0)
                nc.scalar.activation(dA0[:, sl], dt_bf[:, sl], AF.Exp,
                                     scale=acol[:, 0:1])
                nc.scalar.activation(dA1[:, sl], dt_bf[:, sl], AF.Exp,
                                     scale=acol[:, 1:2])
                nc.vector.tensor_tensor(v[:, sl], dt_bf[:, sl], xs[:, sl],
                                        OP.mult)
                nc.vector.tensor_tensor(u0[:, sl], v[:, sl], b0[:, sl],
                                        OP.mult)
                nc.vector.tensor_tensor_scan(
                    h0[:, sl], dA0[:, sl], u0[:, sl],
                    0.0 if j == 0 else h0[:, j * sub - 1:j * sub],
                    OP.mult, OP.add)
            d["un"], d["xs"], d["zs"] = un, xs, zs
            d["bcd"], d["dt"], d["v"] = bcd, dt_bf, v
            d["h0"] = h0
            # rest of bcd rows, remaining dA tiles, broadcasts, u_1..u_3
            nc.sync.dma_start(bcd[0:1, Tc:2 * Tc],
                              bcdt[RANK + 1:RANK + 2, :])
            nc.sync.dma_start(bcd[1:NSTATE, :],
                              bcdt[RANK + 2:RANK + 2 * NSTATE, :])
            dAs = [dA0, dA1]
            for n in range(2, NSTATE):
                dA = dap.tile([C, Tc], BF16, tag="dA", name="dA")
                nc.scalar.activation(dA[:], dt_bf[:], AF.Exp,
                                     scale=acol[:, n:n + 1])
                dAs.append(dA)
            d["dAs"] = dAs
            bcdf = bcd.tensor.reshape([1, NSTATE * 2 * Tc])
            prs = []
            for n in range(NSTATE - 1):
                pr = bcrp.tile([C, 2 * Tc], BF16, tag="bcr", name="bcr")
                nc.sync.dma_start(
                    pr[:],
                    bcdf[0:1, (2 * n + 1) * Tc:(2 * n + 3) * Tc]
                    .broadcast_to([C, 2 * Tc]))
                prs.append(pr)
            c15 = b0p.tile([C, Tc], BF16, tag="c15", name="c15")
            nc.sync.dma_start(
                c15[:],
                bcdf[0:1, (2 * NSTATE - 1) * Tc:2 * NSTATE * Tc]
                .broadcast_to([C, Tc]))
            d["prs"], d["c15"], d["b0"] = prs, c15, b0
            d["us"] = [None] * NSTATE
            d["us"][0] = u0
            for n in range(1, 4):
                emit_u(0, n)

        def emit_scan_loop(k):
            d = P[k]
            last = k + 1 >= nchunk
            ysum = ps_y.tile([C, Tc], F32, tag="ysum", name="ysum")
            d["ysum"] = ysum
            for n in range(NSTATE):
                # ---- hooks: pipeline chunk k+1 prefix / chunk k-1 readout
                if n == 1 and k > 0:
                    emit_readout(k - 1)
                if not last:
                    if n == 1:
                        emit_in_dma(k + 1)
                    elif n == 3:
                        emit_ln_a(k + 1)
                    elif n == 6:
                        emit_ln_b(k + 1)
                    elif n == 8:
                        emit_conv(k + 1)
                    elif n == 10:
                        emit_proj(k + 1)
                    elif n == 11:
                        emit_dA(k + 1)
                    elif n == 12:
                        emit_v(k + 1)
                    elif n >= 13:
                        emit_u(k + 1, n - 13)  # u_0..u_2 of k+1

                # ---- chunk k state n
                dA = d["dAs"][n]
                u = d["us"][n]
                init = 0.0 if k == 0 else carries[n][:]
                if last and n == NSTATE - 1:
                    # final state: chained sub-scans so the readout of sub j
                    # overlaps the scan of sub j+1
                    t0 = k * Tc
                    h = hp.tile([C, Tc], BF16, tag="h", name="h")
                    p = pp.tile([C, Tc], BF16, tag="p", name="p")
                    cb = d["c15"]
                    for j in range(nsub):
                        sl = slice(j * sub, (j + 1) * sub)
                        nc.vector.tensor_tensor_scan(
                            h[:, sl], dA[:, sl], u[:, sl],
                            init if j == 0 else h[:, j * sub - 1:j * sub],
                            OP.mult, OP.add)
                        nc.vector.tensor_tensor(p[:, sl], h[:, sl],
                                                cb[:, sl], OP.mult)
                        nc.tensor.matmul(ysum[:, sl], idn[:], p[:, sl],
                                         start=False, stop=False)
                        nc.tensor.matmul(ysum[:, sl], diag_d[:],
                                         d["xs"][:, sl],
                                         start=False, stop=True)
                        yc = ygp.tile([C, sub], BF16, tag="yc", name="yc")
                        nc.scalar.copy(yc[:], ysum[:, sl])
                        yg = ygp.tile([C, sub], BF16, tag="yg", name="yg")
                        nc.vector.tensor_tensor(yg[:], yc[:],
                                                d["zs"][:, sl], OP.mult)
                        o_ps = ps_b.tile([C, sub], F32, tag="mmb",
                                         name="o_ps")
                        nc.tensor.matmul(o_ps[:], woutT[:], yg[:],
                                         start=True, stop=False)
                        nc.tensor.matmul(o_ps[:], idnf[:], d["xin"][:, sl],
                                         start=False, stop=True)
                        ob = ygp.tile([C, sub], F32, tag="ob", name="ob")
                        nc.scalar.copy(ob[:], o_ps[:])
                        nc.sync.dma_start(
                            y_out[:, t0 + j * sub:t0 + (j + 1) * sub],
                            ob[:])
                else:
                    h = hp.tile([C, Tc], BF16, tag="h", name="h")
                    nc.vector.tensor_tensor_scan(h[:], dA[:], u[:], init,
                                                 OP.mult, OP.add)
                    if not last:
                        nc.vector.tensor_copy(carries[n][:],
                                              h[:, Tc - 1:Tc])
                    p = pp.tile([C, Tc], BF16, tag="p", name="p")
                    cb = d["c15"][:] if n == NSTATE - 1 \
                        else d["prs"][n][:, 0:Tc]
                    nc.vector.tensor_tensor(p[:], h[:], cb, OP.mult)
                    for j in range(nsub):
                        sl = slice(j * sub, (j + 1) * sub)
                        nc.tensor.matmul(ysum[:, sl], idn[:], p[:, sl],
                                         start=(n == 0), stop=False)
                if n + 4 < NSTATE:
                    emit_u(k, n + 4)  # +4 lookahead within chunk k
            if not last:
                emit_u(k + 1, 3)
                for j in range(nsub):
                    sl = slice(j * sub, (j + 1) * sub)
                    nc.tensor.matmul(ysum[:, sl], diag_d[:], d["xs"][:, sl],
                                     start=False, stop=True)

        # ---- bootstrap chunk 0 prefix, then pipelined chunk loop ----
        P[0] = P0
        if True:
            emit_ln_a(0)
            emit_ln_b(0)
            emit_conv(0)
            emit_proj(0)
            emit_dA(0)
            emit_v(0)
            P[0]["h0"] = None
            for n in range(4):
                emit_u(0, n)
        else:
            emit_prefix0()
        for k in range(nchunk):
            emit_scan_loop(k)

    nc.insert_act_table_loads = types.MethodType(_two_act_tables, nc)
    nc.compile()
    return nc


def prep_weights(ln_w, ln_b, in_proj_w, conv_w, conv_b, x_proj_w,
                 dt_proj_w, dt_proj_b, A_log, D, out_proj_w):
    eps = np.full_like(ln_w, LN_EPS)
    z = np.zeros_like(ln_w)
    # fold ln_w into in_proj rows, ln_b into conv/z biases (exact)
    ipx, ipz = in_proj_w[:128], in_proj_w[128:]
    bvec = ipx @ ln_b                    # per-out-channel ln_b feedthrough
    conv_b2 = conv_b + bvec * conv_w.sum(1)
    zb = ipz @ ln_b
    corr = np.stack([-bvec * conv_w[:, :3 - t].sum(1) for t in range(3)],
                    axis=1)              # undo folded bias at t<DCONV-1
    cols = np.stack([ln_w, ln_b, conv_b2, dt_proj_b, D, zb,
                     corr[:, 0], corr[:, 1], corr[:, 2], eps,
                     -conv_b2, -zb, z], axis=1).astype(np.float32)
    return {
        "w_inT": np.ascontiguousarray(np.concatenate(
            [ipx.T * ln_w[:, None] * conv_w[:, kk][None, :]
             for kk in range(4)] + [ipz.T * ln_w[:, None]],
            axis=1).astype(ml_dtypes.bfloat16)),
        "w_xpT": np.ascontiguousarray(
            x_proj_w[list(range(8))
                     + [8 + (i // 2) + 16 * (i % 2) for i in range(32)]].T
            .astype(ml_dtypes.bfloat16)),
        "w_dtT": np.ascontiguousarray(dt_proj_w.T.astype(ml_dtypes.bfloat16)),
        "w_outT": np.ascontiguousarray(
            out_proj_w.T.astype(ml_dtypes.bfloat16)),
        "ident": np.eye(C, dtype=ml_dtypes.bfloat16),
        "nident": (-np.eye(C)).astype(ml_dtypes.bfloat16),
        "diag_d": np.ascontiguousarray(
            np.diag(D).astype(ml_dtypes.bfloat16)),
        "identf": np.eye(C, dtype=np.float32),
        "cols": cols,
        "a_cols": np.ascontiguousarray(-np.exp(A_log.astype(np.float32))),
    }


def kernel(input, ln_w, ln_b, in_proj_w, conv_w, conv_b, x_proj_w,
           dt_proj_w, dt_proj_b, A_log, D, out_proj_w, _run=None):
    input = np.asarray(input, np.float32)
    b, c, H, W = input.shape
    L = H * W
    assert c == C and b == 8
    wts = prep_weights(
        np.asarray(ln_w, np.float32), np.asarray(ln_b, np.float32),
        np.asarray(in_proj_w, np.float32), np.asarray(conv_w, np.float32),
        np.asarray(conv_b, np.float32), np.asarray(x_proj_w, np.float32),
        np.asarray(dt_proj_w, np.float32), np.asarray(dt_proj_b, np.float32),
        np.asarray(A_log, np.float32), np.asarray(D, np.float32),
        np.asarray(out_proj_w, np.float32))
    nc = build_nc(L, 1536, 512)
    in_maps = []
    for i in range(8):
        m = {"x": np.ascontiguousarray(input[i].reshape(c, L))}
        m.update(wts)
        in_maps.append(m)
    run = _run or run_bass_kernel_spmd
    res = run(nc, in_maps, core_ids=list(range(8)))
    out = np.stack([np.asarray(res.results[i]["y"]).reshape(c, H, W)
                    for i in range(8)])
    return out.astype(np.float32)
